# revision 16
# baseline (speedup 1.0000x reference)
"""CSPLayer (GNN message passing) Trainium2 Bass kernel, 8-core SPMD.

Self-contained: hardcodes shapes/sharding for the nn_CSPLayer problem
(N=50000 nodes, E=800000 edges, H=128, B=1000 crystals, 8 cores).

Strategy (graph/data parallel):
- nodes partitioned into 8 contiguous, 128-aligned ranges balanced by edge
  count; edges assigned to the core owning their dst.
- per-node linear maps folded on host into gather tables:
    A = x @ mn_w1[x_dst part] + mn_b1 + ee_b2 @ mn_w1[ef part]
    Bt = x @ mn_w1[x_src part]          (pair-packed rows for int16 gather)
- device phase 1 (per 1024-edge window): dma_gather A[dst], B[src]; edge
  encoder MLP (feature-major); h1 assembled in PSUM via matmuls
  (efc + identity-injects); SiLU; onehot (is_equal) scatter-matmul
  -> node-major aggH; batched dma_scatter_add into HBM slab.
- device phase 2: per 128-node tile: mn_w2 on aggregated h1s (+ deg*b2),
  update MLP, residual, crystal partial sums (onehot matmul), LayerNorm.
- host tail: lattice branch (tiny) + crystal-sum combination.
"""

import numpy as np
import ml_dtypes

import concourse.bass as bass
import concourse.bacc as bacc
import concourse.tile as tile
from concourse import mybir
from concourse import bass_utils
from concourse.masks import make_identity

BF16 = mybir.dt.bfloat16
F32 = mybir.dt.float32
I16 = mybir.dt.int16
U8 = mybir.dt.uint8

P = 128
H = 128
EDGE_DIM = 20
N_NODES = 50000
N_EDGES = 800000
NB = 1000
EPS = 1e-5
NCORES = 8

CALLS = 112          # 1024-edge windows per core (static)
WIN_E = 1024         # edges per window/call
CHUNKS = 8           # 128-edge chunks per call
NTILES = 52          # 128-node tiles per core (static)
NLOC = NTILES * P    # local node slots
DUMP = NLOC          # dump row in slab
SLAB_ROWS = NLOC + P
SBATCH = 8           # windows per dma_scatter_add
NSB = CALLS // SBATCH
EA_MEGA = 16         # calls per edge_attr staging load
B2ROWS = 25088       # ceil(50048/2) pair-packed B rows
BWIN = 3             # crystal windows per core (3*128 = 384 crystals max)

_CACHE = {}


def _bcast_inner(ap, n):
    return bass.AP(tensor=ap.tensor, offset=ap.offset, ap=list(ap.ap) + [[0, n]])


import os
BISECT = os.environ.get('KBISECT', 'all')


def build_nc():
    nc = bacc.Bacc("TRN2", target_bir_lowering=False, debug=False,
                   enable_asserts=False, num_devices=NCORES)
    d = {}
    def inp(name, shape, dt):
        d[name] = nc.dram_tensor(name, shape, dt, kind="ExternalInput")
        return d[name]
    A = inp("A", [NLOC, H], BF16)
    B2 = inp("B2", [B2ROWS, 2 * H], BF16)
    EAT = inp("EAT", [EDGE_DIM, CALLS * WIN_E], BF16)
    AIDX = inp("AIDX", [P, CALLS * WIN_E // 16], I16)
    BIDX = inp("BIDX", [P, CALLS * WIN_E // 16], I16)
    SIDX = inp("SIDX", [P, NSB * SBATCH * P // 16], I16)
    DLOC = inp("DLOC", [P, CALLS * CHUNKS], F32)
    MASK = inp("MASK", [P, CALLS * CHUNKS], U8)
    DEG = inp("DEG", [1, NLOC], BF16)
    BLOC = inp("BLOC", [P, NTILES], F32)
    XT = inp("XT", [H, NLOC], BF16)
    XL = inp("XL", [NLOC, H], F32)
    IOTA = inp("IOTA", [P], BF16)
    EEW1 = inp("EEW1", [EDGE_DIM, H], BF16)
    EEB1 = inp("EEB1", [H], F32)
    W2C = inp("W2C", [H, H], BF16)
    MNW2 = inp("MNW2", [H, H], BF16)
    MNB2 = inp("MNB2", [1, H], BF16)
    UNW1 = inp("UNW1", [2 * H, H], BF16)
    UNB1 = inp("UNB1", [H], F32)
    UNW2 = inp("UNW2", [H, H], BF16)
    UNB2 = inp("UNB2", [1, H], BF16)
    LNG = inp("LNG", [H], F32)
    LNB = inp("LNB", [H], F32)
    XOUT = nc.dram_tensor("XOUT", [NLOC, H], F32, kind="ExternalOutput")
    CSOUT = nc.dram_tensor("CSOUT", [BWIN * P, H], F32, kind="ExternalOutput")
    SLAB = nc.dram_tensor("SLAB", [SLAB_ROWS, H], BF16, kind="Internal")

    with tile.TileContext(nc) as tc:
        import contextlib
        with contextlib.ExitStack() as ctx:
            const = ctx.enter_context(tc.tile_pool(name="const", bufs=1))
            big = ctx.enter_context(tc.tile_pool(name="big", bufs=1))

            # ---------- constants ----------
            def load_col(src, name, n=H, dt=F32):
                t = const.tile([n, 1], dt, tag=name)
                nc.sync.dma_start(out=t[:], in_=src.ap()[:, None])
                return t
            eeb1_c = load_col(EEB1, "eeb1c")
            unb1_c = load_col(UNB1, "unb1c")
            iota_b = const.tile([P, P], BF16)
            nc.gpsimd.dma_start(
                out=iota_b[:], in_=bass.AP(tensor=IOTA, offset=0, ap=[[0, P], [1, P]]))
            lng_b = const.tile([P, H], F32)
            nc.gpsimd.dma_start(
                out=lng_b[:], in_=bass.AP(tensor=LNG, offset=0, ap=[[0, P], [1, H]]))
            lnb_b = const.tile([P, H], F32)
            nc.gpsimd.dma_start(
                out=lnb_b[:], in_=bass.AP(tensor=LNB, offset=0, ap=[[0, P], [1, H]]))
            ident = const.tile([P, P], BF16)
            make_identity(nc, ident[:])
            ones_row = const.tile([1, P], BF16)
            nc.vector.memset(ones_row[:], 1.0)
            eps_c = const.tile([P, 1], F32)
            nc.vector.memset(eps_c[:], EPS)
            zero_t = const.tile([P, H], BF16)
            nc.vector.memset(zero_t[:], 0.0)

            def wtile(src, shape, name):
                t = const.tile(shape, BF16, tag=name)
                nc.sync.dma_start(out=t[:], in_=src.ap())
                return t
            eew1_t = wtile(EEW1, [EDGE_DIM, H], "eew1")
            w2c_t = wtile(W2C, [H, H], "w2c")
            mnw2_t = wtile(MNW2, [H, H], "mnw2")
            mnb2_t = wtile(MNB2, [1, H], "mnb2")
            unw1a_t = const.tile([H, H], BF16, tag="unw1a")
            nc.sync.dma_start(out=unw1a_t[:], in_=UNW1.ap()[0:H, :])
            unw1b_t = const.tile([H, H], BF16, tag="unw1b")
            nc.sync.dma_start(out=unw1b_t[:], in_=UNW1.ap()[H:2 * H, :])
            unw2_t = wtile(UNW2, [H, H], "unw2")
            unb2_t = wtile(UNB2, [1, H], "unb2")

            # ---------- big staged inputs ----------
            aidx_t = big.tile([P, CALLS * WIN_E // 16], I16)
            nc.sync.dma_start(out=aidx_t[:], in_=AIDX.ap())
            bidx_t = big.tile([P, CALLS * WIN_E // 16], I16)
            nc.sync.dma_start(out=bidx_t[:], in_=BIDX.ap())
            sidx_t = big.tile([P, NSB * SBATCH * P // 16], I16)
            nc.sync.dma_start(out=sidx_t[:], in_=SIDX.ap())
            dloc_t = big.tile([P, CALLS * CHUNKS], F32)
            nc.sync.dma_start(out=dloc_t[:], in_=DLOC.ap())
            mask_t = big.tile([P, CALLS * CHUNKS], U8)
            nc.sync.dma_start(out=mask_t[:], in_=MASK.ap())
            deg_t = big.tile([1, NLOC], BF16)
            nc.sync.dma_start(out=deg_t[:], in_=DEG.ap())
            bloc_t = big.tile([P, NTILES], F32)
            nc.sync.dma_start(out=bloc_t[:], in_=BLOC.ap())
            x2slab = big.tile([P, NTILES * P], F32)

            # zero-init SLAB (gpsimd queue, before any scatter_add)
            zt_ap = zero_t[:]
            nc.gpsimd.dma_start(
                out=SLAB.ap().rearrange("(t p) h -> p t h", p=P),
                in_=bass.AP(tensor=zt_ap.tensor, offset=zt_ap.offset,
                            ap=[list(zt_ap.ap)[0], [0, SLAB_ROWS // P],
                                list(zt_ap.ap)[1]]))
            tc.strict_bb_all_engine_barrier()

            # ================= PHASE 1: edges =================
            with contextlib.ExitStack() as p1:
                sb = p1.enter_context(tc.tile_pool(name="p1sb", bufs=3))
                eap = p1.enter_context(tc.tile_pool(name="p1ea", bufs=2))
                stgp = p1.enter_context(tc.tile_pool(name="p1stg", bufs=2))
                pse = p1.enter_context(tc.tile_pool(name="p1pse", bufs=1, space="PSUM"))
                psh = p1.enter_context(tc.tile_pool(name="p1psh", bufs=4, space="PSUM"))
                pss = p1.enter_context(tc.tile_pool(name="p1pss", bufs=1, space="PSUM"))


                ea_mega = None
                staging = None
                for w in range(CALLS):
                    if w % EA_MEGA == 0:
                        ea_mega = eap.tile([EDGE_DIM, EA_MEGA * WIN_E], BF16, tag="ea")
                        nc.sync.dma_start(
                            out=ea_mega[:],
                            in_=EAT.ap()[:, w * WIN_E:(w + EA_MEGA) * WIN_E])
                    if w % SBATCH == 0:
                        staging = stgp.tile([P, SBATCH, H], BF16, tag="stg")
                    ea_sl = ea_mega[:, (w % EA_MEGA) * WIN_E:(w % EA_MEGA + 1) * WIN_E]

                    ag = sb.tile([P, CHUNKS, H], BF16, tag="ag")
                    nc.gpsimd.dma_gather(
                        out_ap=ag[:], in_ap=A.ap(),
                        idxs_ap=aidx_t[:, w * 64:(w + 1) * 64],
                        num_idxs=WIN_E, num_idxs_reg=WIN_E, elem_size=H)
                    bg = sb.tile([P, CHUNKS, 2 * H], BF16, tag="bg")
                    nc.gpsimd.dma_gather(
                        out_ap=bg[:], in_ap=B2.ap(),
                        idxs_ap=bidx_t[:, w * 64:(w + 1) * 64],
                        num_idxs=WIN_E, num_idxs_reg=WIN_E, elem_size=2 * H)
                    bsel = sb.tile([P, CHUNKS, H], BF16, tag="bsel")
                    nc.vector.tensor_copy(out=bsel[:], in_=bg[:, :, 0:H])
                    nc.vector.copy_predicated(
                        out=bsel[:],
                        mask=_bcast_inner(mask_t[:, w * CHUNKS:(w + 1) * CHUNKS], H),
                        data=bg[:, :, H:2 * H])

                    # edge encoder layer 1 (feature-major)
                    eh1p = pse.tile([P, WIN_E], F32, space="PSUM", tag="eh1")
                    nc.tensor.matmul(out=eh1p[:, 0:512], lhsT=eew1_t[:],
                                     rhs=ea_sl[:, 0:512], start=True, stop=True)
                    nc.tensor.matmul(out=eh1p[:, 512:1024], lhsT=eew1_t[:],
                                     rhs=ea_sl[:, 512:1024], start=True, stop=True)
                    eh1s = sb.tile([P, WIN_E], BF16, tag="eh1s")
                    nc.scalar.activation(out=eh1s[:], in_=eh1p[:],
                                         func=mybir.ActivationFunctionType.Silu,
                                         bias=eeb1_c[:])

                    # h1 = efc + A[dst] + B[src] per chunk, in PSUM
                    h1pa = psh.tile([P, 512], F32, space="PSUM", tag="h1")
                    h1pb = psh.tile([P, 512], F32, space="PSUM", tag="h1")
                    for c in range(CHUNKS):
                        tp = h1pa if c < 4 else h1pb
                        sl = tp[:, (c % 4) * H:(c % 4 + 1) * H]
                        nc.tensor.matmul(out=sl, lhsT=eh1s[:, c * H:(c + 1) * H],
                                         rhs=w2c_t[:], start=(c % 4 == 0), stop=False)
                        nc.tensor.matmul(out=sl, lhsT=ident[:], rhs=ag[:, c, :],
                                         start=False, stop=False)
                        nc.tensor.matmul(out=sl, lhsT=ident[:], rhs=bsel[:, c, :],
                                         start=False, stop=(c % 4 == 3))
                    h1s = sb.tile([P, WIN_E], BF16, tag="h1s")
                    nc.scalar.activation(out=h1s[:, 0:512], in_=h1pa[:],
                                         func=mybir.ActivationFunctionType.Silu)
                    nc.scalar.activation(out=h1s[:, 512:1024], in_=h1pb[:],
                                         func=mybir.ActivationFunctionType.Silu)

                    # onehot + scatter matmuls -> aggH [n, k] for this window
                    oh = sb.tile([P, CHUNKS, P], BF16, tag="oh")
                    for c in range(CHUNKS):
                        nc.vector.tensor_scalar(
                            out=oh[:, c, :], in0=iota_b[:],
                            scalar1=dloc_t[:, w * CHUNKS + c:w * CHUNKS + c + 1],
                            scalar2=None, op0=mybir.AluOpType.is_equal)
                    scp = pss.tile([P, P], F32, space="PSUM", tag="scp")
                    for c in range(CHUNKS):
                        nc.tensor.matmul(out=scp[:], lhsT=oh[:, c, :],
                                         rhs=h1s[:, c * H:(c + 1) * H],
                                         start=(c == 0), stop=(c == CHUNKS - 1))
                    nc.vector.tensor_copy(out=staging[:, w % SBATCH, :], in_=scp[:])
                    if w % SBATCH == SBATCH - 1:
                        wb = w // SBATCH
                        nc.gpsimd.dma_scatter_add(
                            out_ap=SLAB.ap(), in_ap=staging[:],
                            idxs_ap=sidx_t[:, wb * 64:(wb + 1) * 64],
                            num_idxs=SBATCH * P, num_idxs_reg=SBATCH * P,
                            elem_size=H)

            tc.strict_bb_all_engine_barrier()

            # ================= PHASE 2a: nodes =================
            csp = ctx.enter_context(tc.tile_pool(name="csp", bufs=1, space="PSUM"))
            cs_ps = []
            for i in range(BWIN):
                cs_tile = csp.tile([P, H], F32, space="PSUM", tag=f"cs{i}")
                cs_ps.append(cs_tile)
            with contextlib.ExitStack() as p2:
                sb2 = p2.enter_context(tc.tile_pool(name="p2sb", bufs=3))
                ps2 = p2.enter_context(tc.tile_pool(name="p2ps", bufs=2, space="PSUM"))
                for t in range(NTILES):
                    nsl = slice(t * P, (t + 1) * P)
                    aggh = sb2.tile([P, H], BF16, tag="aggh")
                    nc.sync.dma_start(out=aggh[:], in_=SLAB.ap()[nsl, :])
                    agghT_ps = ps2.tile([P, P], BF16, space="PSUM", tag="p2t")
                    nc.tensor.transpose(out=agghT_ps[:], in_=aggh[:], identity=ident[:])
                    agghT = sb2.tile([P, P], BF16, tag="agghT")
                    nc.vector.tensor_copy(out=agghT[:], in_=agghT_ps[:])

                    aggrT_ps = ps2.tile([P, P], F32, space="PSUM", tag="p2")
                    nc.tensor.matmul(out=aggrT_ps[:], lhsT=mnw2_t[:], rhs=agghT[:],
                                     start=True, stop=False)
                    nc.tensor.matmul(out=aggrT_ps[:], lhsT=mnb2_t[:],
                                     rhs=deg_t[:, nsl], start=False, stop=True)
                    aggrT = sb2.tile([P, P], BF16, tag="aggrT")
                    nc.vector.tensor_copy(out=aggrT[:], in_=aggrT_ps[:])

                    xT_t = sb2.tile([H, P], BF16, tag="xT")
                    nc.sync.dma_start(out=xT_t[:], in_=XT.ap()[:, nsl])
                    h1u_ps = ps2.tile([P, P], F32, space="PSUM", tag="p2")
                    nc.tensor.matmul(out=h1u_ps[:], lhsT=unw1a_t[:], rhs=xT_t[:],
                                     start=True, stop=False)
                    nc.tensor.matmul(out=h1u_ps[:], lhsT=unw1b_t[:],
                                     rhs=aggrT[:], start=False, stop=True)
                    h1us = sb2.tile([P, P], BF16, tag="h1us")
                    nc.scalar.activation(out=h1us[:], in_=h1u_ps[:],
                                         func=mybir.ActivationFunctionType.Silu,
                                         bias=unb1_c[:])
                    upd_ps = ps2.tile([P, P], F32, space="PSUM", tag="p2")
                    nc.tensor.matmul(out=upd_ps[:], lhsT=h1us[:], rhs=unw2_t[:],
                                     start=True, stop=False)
                    nc.tensor.matmul(out=upd_ps[:], lhsT=ones_row[:], rhs=unb2_t[:],
                                     start=False, stop=True)

                    xl_t = sb2.tile([P, H], F32, tag="xl")
                    nc.sync.dma_start(out=xl_t[:], in_=XL.ap()[nsl, :])
                    nc.vector.tensor_tensor(out=x2slab[:, nsl], in0=upd_ps[:],
                                            in1=xl_t[:], op=mybir.AluOpType.add)
                    x2b = sb2.tile([P, H], BF16, tag="x2b")
                    nc.vector.tensor_copy(out=x2b[:], in_=x2slab[:, nsl])
                    for bw in range(BWIN):
                        ohb = sb2.tile([P, P], BF16, tag="ohb")
                        # ohb[n, j] = (bloc[n] - 128*bw == j) <=> iota[j] - bloc[n] == -128*bw
                        nc.vector.tensor_scalar(
                            out=ohb[:], in0=iota_b[:],
                            scalar1=bloc_t[:, t:t + 1],
                            scalar2=float(-128 * bw),
                            op0=mybir.AluOpType.subtract,
                            op1=mybir.AluOpType.is_equal)
                        nc.tensor.matmul(out=cs_ps[bw][:], lhsT=ohb[:], rhs=x2b[:],
                                         start=(t == 0), stop=(t == NTILES - 1))

            tc.strict_bb_all_engine_barrier()

            # ================= PHASE 2b: LayerNorm =================
            with contextlib.ExitStack() as p3:
                sb3 = p3.enter_context(tc.tile_pool(name="p3sb", bufs=3))
                for t in range(NTILES):
                    nsl = slice(t * P, (t + 1) * P)
                    stats = sb3.tile([P, 6], F32, tag="st")
                    nc.vector.bn_stats(out=stats[:], in_=x2slab[:, nsl])
                    mv = sb3.tile([P, 2], F32, tag="mv")
                    nc.vector.bn_aggr(out=mv[:], in_=stats[:])
                    sdev = sb3.tile([P, 1], F32, tag="sd")
                    nc.scalar.activation(out=sdev[:], in_=mv[:, 1:2],
                                         func=mybir.ActivationFunctionType.Sqrt,
                                         bias=eps_c[:])
                    rstd = sb3.tile([P, 1], F32, tag="rs")
                    nc.vector.reciprocal(out=rstd[:], in_=sdev[:])
                    xn = sb3.tile([P, H], F32, tag="xn")
                    nc.vector.tensor_scalar(
                        out=xn[:], in0=x2slab[:, nsl],
                        scalar1=mv[:, 0:1], scalar2=rstd[:, 0:1],
                        op0=mybir.AluOpType.subtract, op1=mybir.AluOpType.mult)
                    y = sb3.tile([P, H], F32, tag="y")
                    nc.vector.tensor_tensor(out=y[:], in0=xn[:], in1=lng_b[:],
                                            op=mybir.AluOpType.mult)
                    nc.vector.tensor_tensor(out=y[:], in0=y[:], in1=lnb_b[:],
                                            op=mybir.AluOpType.add)
                    nc.sync.dma_start(out=XOUT.ap()[nsl, :], in_=y[:])
                # crystal sums out
                for bw in range(BWIN):
                    cs_sb = sb3.tile([P, H], F32, tag="cso")
                    nc.vector.tensor_copy(out=cs_sb[:], in_=cs_ps[bw][:])
                    nc.sync.dma_start(
                        out=CSOUT.ap()[bw * P:(bw + 1) * P, :], in_=cs_sb[:])

    nc.compile()
    return nc


# ====================== host preparation ======================

def _wrap16(idx, ncols_per_call=None):
    """idx [n] int -> [128, n/16] int16 wrapped layout (16-part, replicated 8x)."""
    n = idx.shape[0]
    blk = idx.astype(np.int16).reshape(n // 16, 16).T  # [16, n/16]
    return np.tile(blk, (8, 1))


def host_prep(node_features, edge_index, edge_attr, lattice, batch,
              ee_w1, ee_b1, ee_w2, ee_b2,
              mn_w1, mn_b1, mn_w2, mn_b2,
              un_w1, un_b1, un_w2, un_b2,
              l2a_w, l2a_b, a2l_w, a2l_b,
              node_ln_g, node_ln_b, lat_ln_g, lat_ln_b):
    f32 = np.float32
    bf16 = ml_dtypes.bfloat16
    x = np.asarray(node_features, f32)
    ei = np.asarray(edge_index)
    src = ei[0].astype(np.int64)
    dst = ei[1].astype(np.int64)
    ea = np.asarray(edge_attr, f32)
    lat9 = np.asarray(lattice, f32).reshape(NB, 9)
    bat = np.asarray(batch).astype(np.int64)

    w1a = np.asarray(mn_w1, f32)[0:H]
    w1b = np.asarray(mn_w1, f32)[H:2 * H]
    w1c = np.asarray(mn_w1, f32)[2 * H:3 * H]
    A_all = x @ w1a + np.asarray(mn_b1, f32) + np.asarray(ee_b2, f32) @ w1c
    B_all = x @ w1b
    W2c = np.asarray(ee_w2, f32) @ w1c
    latb = (lat9 @ np.asarray(l2a_w, f32) + np.asarray(l2a_b, f32))[bat]
    xl_all = x + latb

    # B pair-packed table
    B_pad = np.zeros((B2ROWS * 2, H), f32)
    B_pad[:N_NODES] = B_all
    B2 = B_pad.reshape(B2ROWS, 2 * H).astype(bf16)

    # core boundaries: 128-aligned, edge-balanced
    deg_all = np.bincount(dst, minlength=N_NODES)
    cum = np.concatenate([[0], np.cumsum(deg_all)])
    bounds = [0]
    for c in range(1, NCORES):
        target = N_EDGES * c // NCORES
        nb_ = int(np.searchsorted(cum, target))
        nb_ = max(128, min(N_NODES - 128, ((nb_ + 63) // 128) * 128))
        bounds.append(max(nb_, bounds[-1] + 128))
    bounds.append(N_NODES)

    order = np.argsort(dst, kind="stable")
    dst_s = dst[order]
    core_edge_lo = np.searchsorted(dst_s, bounds[:-1])
    core_edge_hi = np.searchsorted(dst_s, bounds[1:])

    in_maps = []
    meta = []
    const_common = {
        "B2": B2,
        "IOTA": np.arange(P).astype(bf16),
        "EEW1": np.asarray(ee_w1, f32).astype(bf16),
        "EEB1": np.asarray(ee_b1, f32),
        "W2C": W2c.astype(bf16),
        "MNW2": np.asarray(mn_w2, f32).astype(bf16),
        "MNB2": np.asarray(mn_b2, f32).astype(bf16).reshape(1, H),
        "UNW1": np.asarray(un_w1, f32).astype(bf16),
        "UNB1": np.asarray(un_b1, f32),
        "UNW2": np.asarray(un_w2, f32).astype(bf16),
        "UNB2": np.asarray(un_b2, f32).astype(bf16).reshape(1, H),
        "LNG": np.asarray(node_ln_g, f32),
        "LNB": np.asarray(node_ln_b, f32),
    }

    for c in range(NCORES):
        nbase, nend = bounds[c], bounds[c + 1]
        nreal = nend - nbase
        assert nreal <= NLOC, (c, nreal)
        e_ids = order[core_edge_lo[c]:core_edge_hi[c]]
        E_c = e_ids.shape[0]
        d_loc = (dst[e_ids] - nbase).astype(np.int64)  # sorted ascending
        s_glob = src[e_ids].astype(np.int64)
        deg_loc = np.bincount(d_loc, minlength=NLOC)[:NLOC]

        # windows: consecutive nodes, <=1024 edges, <=128 nodes each
        win_node_lo, win_node_hi, win_edge_lo = [], [], []
        n0 = 0
        e_lo = 0
        node_edge_cum = np.concatenate([[0], np.cumsum(deg_loc[:nreal])])
        while n0 < nreal:
            n1_cap = min(n0 + P, nreal)
            # max n1 with edges(n0:n1) <= WIN_E
            hi = int(np.searchsorted(node_edge_cum, node_edge_cum[n0] + WIN_E,
                                     side="right")) - 1
            n1 = max(n0 + 1, min(n1_cap, hi))
            win_node_lo.append(n0)
            win_node_hi.append(n1)
            win_edge_lo.append(int(node_edge_cum[n0]))
            n0 = n1
        W_real = len(win_node_lo)
        assert W_real <= CALLS, (c, W_real)
        assert E_c <= CALLS * WIN_E

        aidx = np.zeros((CALLS * WIN_E,), np.int64)
        bidx = np.zeros((CALLS * WIN_E,), np.int64)
        mask = np.zeros((CALLS * WIN_E,), f32)
        dloc_w = np.full((CALLS * WIN_E,), -1.0, f32)
        ea_s = np.zeros((CALLS * WIN_E, EDGE_DIM), f32)
        sidx = np.full((NSB * SBATCH * P,), DUMP, np.int64)

        for w in range(W_real):
            nlo, nhi = win_node_lo[w], win_node_hi[w]
            elo = win_edge_lo[w]
            ehi = int(node_edge_cum[nhi])
            ne = ehi - elo
            assert ne <= WIN_E
            sl = slice(w * WIN_E, w * WIN_E + ne)
            aidx[sl] = d_loc[elo:ehi]
            bidx[sl] = s_glob[elo:ehi] >> 1
            mask[sl] = (s_glob[elo:ehi] & 1).astype(f32)
            dloc_w[sl] = (d_loc[elo:ehi] - nlo).astype(f32)
            ea_s[sl] = ea[e_ids[elo:ehi]]
            cnt = nhi - nlo
            rows = np.full((P,), DUMP, np.int64)
            rows[:cnt] = nlo + np.arange(cnt)
            sidx[w * P:(w + 1) * P] = rows

        # per-call wrapped idx planes
        acols = np.concatenate(
            [_wrap16(aidx[w * WIN_E:(w + 1) * WIN_E]) for w in range(CALLS)], axis=1)
        bcols = np.concatenate(
            [_wrap16(bidx[w * WIN_E:(w + 1) * WIN_E]) for w in range(CALLS)], axis=1)
        scols = np.concatenate(
            [_wrap16(sidx[i * SBATCH * P:(i + 1) * SBATCH * P]) for i in range(NSB)],
            axis=1)
        # [128, CALLS*8] planes: (p, w*8+ch) = v[w*1024 + ch*128 + p]
        dl = dloc_w.reshape(CALLS, CHUNKS, P).transpose(2, 0, 1).reshape(
            P, CALLS * CHUNKS, order="F") if False else \
            np.ascontiguousarray(dloc_w.reshape(CALLS * CHUNKS, P).T)
        mk = np.ascontiguousarray(mask.reshape(CALLS * CHUNKS, P).T)

        xT_c = np.zeros((H, NLOC), f32)
        xT_c[:, :nreal] = x[nbase:nend].T
        xl_c = np.zeros((NLOC, H), f32)
        xl_c[:nreal] = xl_all[nbase:nend]
        A_c = np.zeros((NLOC, H), f32)
        A_c[:nreal] = A_all[nbase:nend]
        deg_c = np.zeros((1, NLOC), f32)
        deg_c[0, :nreal] = deg_loc[:nreal]
        b_base = int(bat[nbase])
        bloc = np.full((NLOC,), -1.0, f32)
        bloc[:nreal] = (bat[nbase:nend] - b_base).astype(f32)
        assert bloc.max() < BWIN * P
        bloc_pl = np.ascontiguousarray(bloc.reshape(NTILES, P).T)

        im = dict(const_common)
        im.update({
            "A": A_c.astype(bf16),
            "EAT": np.ascontiguousarray(ea_s.T).astype(bf16),
            "AIDX": acols, "BIDX": bcols, "SIDX": scols,
            "DLOC": dl, "MASK": mk.astype(np.uint8),
            "DEG": deg_c.astype(bf16),
            "BLOC": bloc_pl,
            "XT": xT_c.astype(bf16),
            "XL": xl_c,
        })
        in_maps.append(im)
        meta.append(dict(nbase=nbase, nreal=nreal, b_base=b_base,
                         b_last=int(bat[nend - 1])))
    return in_maps, meta, dict(x=x, lat9=lat9, bat=bat)


def host_post(results, meta, aux,
              a2l_w, a2l_b, lat_ln_g, lat_ln_b, **_):
    f32 = np.float32
    x_out = np.zeros((N_NODES, H), f32)
    sums = np.zeros((NB, H), f32)
    for c in range(NCORES):
        m = meta[c]
        r = results[c]
        x_out[m["nbase"]:m["nbase"] + m["nreal"]] = r["XOUT"][:m["nreal"]]
        ncr = m["b_last"] - m["b_base"] + 1
        sums[m["b_base"]:m["b_base"] + ncr] += r["CSOUT"][:ncr]
    counts = np.bincount(aux["bat"], minlength=NB).astype(f32)
    atom_info = sums / np.maximum(counts, 1.0)[:, None]
    lat9 = aux["lat9"] + atom_info @ np.asarray(a2l_w, f32) + np.asarray(a2l_b, f32)
    mu = lat9.mean(-1, keepdims=True)
    var = lat9.var(-1, keepdims=True)
    lat9 = (lat9 - mu) / np.sqrt(var + EPS) * np.asarray(lat_ln_g, f32) \
        + np.asarray(lat_ln_b, f32)
    return x_out, lat9.reshape(NB, 3, 3)


def kernel(**inputs):
    if "nc" not in _CACHE:
        _CACHE["nc"] = build_nc()
    nc = _CACHE["nc"]
    in_maps, meta, aux = host_prep(**inputs)
    res = bass_utils.run_bass_kernel_spmd(nc, in_maps,
                                          core_ids=list(range(NCORES)))
    return host_post(res.results, meta, aux,
                     a2l_w=inputs["a2l_w"], a2l_b=inputs["a2l_b"],
                     lat_ln_g=inputs["lat_ln_g"], lat_ln_b=inputs["lat_ln_b"])
